# revision 19
# baseline (speedup 1.0000x reference)
"""AdderVDSR kernel for 8 TRN2 NeuronCores.

Mathematical collapse: every AdderNet block computes
    relu(-sum_{c,kh,kw} |patch - w|)
and the inner sum of 576 absolute values of continuous random quantities is
strictly positive, so each block outputs exactly 0 in fp32.  After the first
adder layer the hidden state is identically zero and stays zero, so

    reference(x, ...) == pixel_shuffle(conv3(x, up_w, up_b), 2) + out_b

bit-exactly.  The kernel therefore only computes the 3->12 channel 3x3
up-conv, the pixel shuffle, and the two bias adds.

Distribution: data-parallel over H; core i computes pre-shuffle rows
[16i, 16i+16) -> output rows [32i, 32i+32).

Device formulation: block-diagonal im2col GEMM over G=8 groups (b x four
4-row blocks).  K = 8 groups x 14 taps = 112, contracted in two accumulating
passes (taps 0..13, then 14..26 + ones/bias row); M = 128 PSUM partitions =
4 shuffle phases (dr,dc) at 32-aligned bases + 8c + g; N = 512 pixels in one
PSUM bank, pipelined as two 256-column halves (4 matmuls).  Pixel-shuffle
interleave copies (24 lanes, phase x col-half, split DVE/ACT) land in a
24-partition SBUF layout whose partitions are 8 KiB DRAM-contiguous slab
rows, so each output DMA fans out across all DMA engines.

The NEFF epilogue (walrus' ~7us all-semaphore reset sweep, gated on the
block-exit barrier) dominates at this scale, so no engine waits for output
DMA completion: the sweep overlaps the output drain.
"""

import numpy as np

import concourse.bass as bass
import concourse.mybir as mybir
from concourse.bass_utils import run_bass_kernel_spmd

N_CORES = 8
B, C, H, W = 2, 3, 128, 128
RH = H // N_CORES          # 16 pre-shuffle rows per core
G = 8                      # groups: (b, 4-row block)
KP = 14                    # taps per pass (27 taps + ones row = 2x14)
KK = G * KP                # 112 matmul contraction rows
M = 128                    # psum partitions: 32*(dr,dc) + 8c + g, 32-aligned
NW = 512                   # pixels per group: 4 rows x 128 w
NH = NW // 2               # matmul column-half
WCOLS = 2 * M              # two passes' stationary weights
XW = WCOLS + 2 * NW        # 1280 total xcol columns

_f32 = mybir.dt.float32
_bf16 = mybir.dt.bfloat16

# rhs column blocks: [weights | p0c0 | p1c0 | p0c1 | p1c1] -- chunk A is
# everything column-half 0 needs (both passes), so the h0 matmul pair and the
# first interleave copies start before chunk B lands.
R1 = WCOLS            # p0c0
R2 = WCOLS + NH       # p1c0
R3 = WCOLS + NW       # p0c1
R4 = WCOLS + NW + NH  # p1c1
CHA = WCOLS + NW


def build_graph():
    nc = bass.Bass()
    xcol = nc.declare_dram_parameter("xcol", [KK, XW], _bf16, isOutput=False)
    out = nc.declare_dram_parameter("out", [B, C, 2 * RH, 2 * W], _f32, isOutput=True)

    with (
        nc.sbuf_tensor([KK, XW], _bf16) as P,
        nc.sbuf_tensor([24, 2048], _f32) as sb_out,
        nc.sbuf_tensor([1, 16], _bf16) as scratch,
        nc.sbuf_tensor([128, 256], _bf16) as wrm,
        nc.psum_tensor([M, 2 * NW], _f32) as pst,
        nc.psum_tensor([1, 256], _f32) as warm,
        nc.semaphore("dma_a") as dma_a,
        nc.semaphore("dma_b") as dma_b,
        nc.semaphore("mm_sem") as mm_sem,
        nc.semaphore("cp_v") as cp_v,
        nc.semaphore("cp_s") as cp_s,
        nc.semaphore("out_sem") as out_sem,
        nc.semaphore("wrm_sem") as wrm_sem,
        nc.Block() as block,
    ):
        # Output DRAM view: partition q = 12b + 4c + rb is the contiguous
        # 8 KiB slab out[b, c, 8*rb : 8*rb+8, :]; h splits it into 4 KiB
        # row-halves matching psum column-halves.
        out_v = out.rearrange(
            "b c (rb h rows) w -> (b c rb) h (rows w)", rb=4, h=2, rows=4
        )

        def copy_ap(e, h):
            dr, dc = e // 2, e % 2
            src = pst[32 * e : 32 * e + 24, NW * h : NW * h + NH].rearrange(
                "p (n w) -> p n w", n=2
            )
            dst = sb_out.rearrange(
                "q (n dr w dc) -> q n dr w dc", n=4, dr=2, w=128, dc=2
            )[:, 2 * h : 2 * h + 2, dr, :, dc]
            return dst, src

        @block.sync
        def _(sync):
            sync.dma_start(out=P[:, :CHA], in_=xcol[:, :CHA]).then_inc(dma_a, 16)
            # Row-half output DMAs chase the interleave copies; nothing waits
            # for their completion -- the NEFF epilogue overlaps the drain.
            sync.wait_ge(cp_v, 2)
            sync.wait_ge(cp_s, 2)
            sync.dma_start(out=out_v[:, 0], in_=sb_out[:, 0:1024]).then_inc(out_sem, 16)
            sync.wait_ge(cp_v, 4)
            sync.wait_ge(cp_s, 4)
            sync.dma_start(out=out_v[:, 1], in_=sb_out[:, 1024:2048]).then_inc(
                out_sem, 16
            )

        @block.gpsimd
        def _(gpsimd):
            gpsimd.memset(wrm[:, :], 1.0).then_inc(wrm_sem, 1)

        @block.tensor
        def _(tensor):
            # PE clock ramps 1.2->2.4 GHz after ~3.4us of activity; dummy
            # matmuls spanning the input-drain window warm it so the real
            # matmuls run at the fast clock.
            ones = nc.const_aps.aps[(mybir.dt.bfloat16, 1.0)]
            tensor.wait_ge(wrm_sem, 1)
            for _ in range(10):
                tensor.matmul(
                    warm[0:1, 0:256], lhsT=ones[0:128, 0:1], rhs=wrm[:, :],
                    start=True, stop=True,
                )
            tensor.wait_ge(dma_a, 16)
            tensor.matmul(
                pst[:, 0:NH], lhsT=P[:, 0:M], rhs=P[:, R1 : R1 + NH],
                start=True, stop=False,
            )
            tensor.matmul(
                pst[:, 0:NH], lhsT=P[:, M:WCOLS], rhs=P[:, R2 : R2 + NH],
                start=False, stop=True,
            ).then_inc(mm_sem, 1)
            tensor.wait_ge(dma_b, 16)
            tensor.matmul(
                pst[:, NW : NW + NH], lhsT=P[:, 0:M], rhs=P[:, R3 : R3 + NH],
                start=True, stop=False,
            )
            tensor.matmul(
                pst[:, NW : NW + NH], lhsT=P[:, M:WCOLS], rhs=P[:, R4 : R4 + NH],
                start=False, stop=True,
            ).then_inc(mm_sem, 1)

        @block.vector
        def _(vector):
            for h in range(2):
                vector.wait_ge(mm_sem, h + 1)
                for e in (0, 1):
                    dst, src = copy_ap(e, h)
                    vector.tensor_copy(dst, src).then_inc(cp_v, 1)

        @block.scalar
        def _(scalar):
            # Chunk B goes out on the ACT HWDGE ring, overlapping chunk A's
            # descriptor generation on the SP ring.
            scalar.dma_start(out=P[:, CHA:], in_=xcol[:, CHA:]).then_inc(dma_b, 16)
            # Dummy tiny copy pulls ACT_TABLE_LOAD off the critical path.
            # Src is a preamble-initialized const tensor (no input dependency).
            ones = nc.const_aps.aps[(mybir.dt.bfloat16, 1.0)]
            scalar.copy(scratch[0:1, 0:1], ones[0:1, 0:1])
            for h in range(2):
                scalar.wait_ge(mm_sem, h + 1)
                for e in (2, 3):
                    dst, src = copy_ap(e, h)
                    scalar.copy(dst, src).then_inc(cp_s, 1)

    return nc


def make_in_maps(x, up_w, up_b, out_b):
    """Shard inputs: per-core block-diagonal im2col + packed weights."""
    import ml_dtypes

    bf16 = ml_dtypes.bfloat16
    x = np.asarray(x, dtype=np.float32)
    up_w = np.asarray(up_w, dtype=np.float32)
    up_b = np.asarray(up_b, dtype=np.float32)
    out_b = np.asarray(out_b, dtype=np.float32)

    xp = np.zeros((B, C, H + 2, W + 2), dtype=np.float32)
    xp[:, :, 1 : H + 1, 1 : W + 1] = x

    # Stationary weights, shared across cores.
    # lhsT_t[14g + kappa, m] = [g == g(m)] * w(tau=14t+kappa; m), block-diag.
    wb = np.zeros((KK, WCOLS), dtype=np.float32)
    for e in range(4):
        dr, dc = e // 2, e % 2
        for c in range(C):
            o = c * 4 + dr * 2 + dc
            for g in range(G):
                b2, rb = divmod(g, 4)
                m = e * 32 + 12 * b2 + 4 * c + rb
                for tau in range(27):
                    c2, kh, kw = tau // 9, (tau // 3) % 3, tau % 3
                    t, kappa = divmod(tau, KP)
                    wb[KP * g + kappa, M * t + m] = up_w[o, c2, kh, kw]
                # tau=27 (t=1, kappa=13): ones-row bias
                wb[KP * g + 13, M + m] = up_b[o] + out_b[c]

    in_maps = []
    for i in range(N_CORES):
        xcol = np.empty((KK, XW), dtype=np.float32)
        xcol[:, :WCOLS] = wb
        pat = np.empty((KK, 2, 4, W), dtype=np.float32)  # [row, t, n, w]
        for g in range(G):
            b, rb = divmod(g, 4)
            r0 = RH * i + 4 * rb
            for kappa in range(KP):
                for t in range(2):
                    tau = KP * t + kappa
                    if tau == 27:
                        pat[KP * g + kappa, t] = 1.0
                    else:
                        c, kh, kw = tau // 9, (tau // 3) % 3, tau % 3
                        pat[KP * g + kappa, t] = xp[
                            b, c, r0 + kh : r0 + kh + 4, kw : kw + W
                        ]
        # column order (h, t, n', w): chunk A = both passes of n-half 0
        xcol[:, WCOLS:] = (
            pat.reshape(KK, 2, 2, 2, W).transpose(0, 2, 1, 3, 4).reshape(KK, 8 * W)
        )
        in_maps.append({"xcol": xcol.astype(bf16)})
    return in_maps


def kernel(x, up_w, up_b, in_w, in_b, adder_w, out_w, out_b):
    nc = build_graph()
    in_maps = make_in_maps(x, up_w, up_b, out_b)
    res = run_bass_kernel_spmd(nc, in_maps, core_ids=list(range(N_CORES)))
    slabs = [np.asarray(res.results[i]["out"]) for i in range(N_CORES)]
    return np.concatenate(slabs, axis=2).astype(np.float32)
